# revision 1
# baseline (speedup 1.0000x reference)
"""Trainium2 Bass kernel for nn_Loss_60430189855357.

BCEWithLogits loss + frame metrics over x[32,4,4000,96] @ W[96] + b.

Strategy (data-parallel over batch, 8 cores):
  - each core gets x[4,4,4000,96] and labels[4,4,4000]
  - on-chip: logits z = sum_f(x*W) + b via DVE multiply + segmented reduce
    (layout: 125 partitions each owning 32 t-rows; f contiguous)
  - softplus(z) accumulated on ACT; z*y, and the 4000-frame metric counts
    (match / label_zero / pred_zero combos) on DVE
  - per-core output: [125, 5] partial sums (softplus, z*y, correct, FA, MS);
    host reduces and applies the reference's sequential normalization.
"""

import os
import sys

import numpy as np

if os.path.isdir("/opt/trn_rl_repo") and "/opt/trn_rl_repo" not in sys.path:
    sys.path.insert(0, "/opt/trn_rl_repo")

B, S, T, F = 32, 4, 4000, 96
NCORES = 8
BSH = B // NCORES  # 4 batches per core
P = 125            # SBUF partitions used (T = P * I)
I = T // P         # 32 t-rows per partition
SEG = I * F        # 3072 contiguous floats per (partition, s)

# acc_out column layout: [softplus, z*y, correct, FA, MS]
ACC_COLS = 5
C_SP, C_ZY, C_CORR, C_FA, C_MS = 0, 1, 2, 3, 4

TRACE = False          # test.py can flip this to get a profiled run
LAST_RESULT = [None]   # test.py reads BassKernelResults from here


def build_nc(bsh=BSH, s_dim=S, t_dim=T, f_dim=F, p_dim=P):
    import concourse.bacc as bacc
    import concourse.mybir as mybir
    from concourse.tile import TileContext
    from concourse.tile_rust import add_dep_helper

    i_dim = t_dim // p_dim
    assert p_dim * i_dim == t_dim
    seg = i_dim * f_dim
    dt = mybir.dt
    Alu = mybir.AluOpType
    Ax = mybir.AxisListType
    Act = mybir.ActivationFunctionType

    nc = bacc.Bacc()
    x_d = nc.declare_dram_parameter("x", [bsh, s_dim, t_dim, f_dim], dt.float32, isOutput=False)
    lab_d = nc.declare_dram_parameter("labels", [bsh, s_dim, t_dim], dt.float32, isOutput=False)
    # wb packs [W row | bias]; the full repeated-W tile is built on-chip by
    # log-doubling copies so the constant DMA is tiny (388 B/partition
    # instead of 12.3 KB, which sat on the critical path)
    wb_d = nc.declare_dram_parameter("wb", [p_dim, f_dim + 1], dt.float32, isOutput=False)
    acc_cols = 5
    c_zy, c_corr, c_fa, c_ms = 0, 1, 2, 3
    acc_d = nc.declare_dram_parameter("acc_out", [p_dim, acc_cols], dt.float32, isOutput=True)

    # partition p owns t-rows [i_dim*p, i_dim*(p+1))
    x_re = x_d[:].rearrange("b s (p i) f -> b s p (i f)", p=p_dim)
    lab_re = lab_d[:].rearrange("b s (p i) -> p b s i", p=p_dim)

    # The NEFF format allows at most ONE sync wait per instruction (Bacc's
    # generate_event_semaphores splits overflow, but only EventSemaphore can
    # hold 2), so the structure keeps every instruction's dependency set
    # small:
    #   - small DMAs (wb, labels, stores) ride HWDGE: <= 8 total so no HWDGE
    #     lane is recycled
    #   - x loads ride SWDGE per (b, s) chunk (1.5 MB each) for fine-grained
    #     overlap; the x-slot WAR dependency is absorbed by a tiny gpsimd
    #     copy (join) so reused-slot DMAs only carry their lane wait
    with (
        TileContext(nc) as tc,
        tc.tile_pool(name="xpool", bufs=8) as px,
        tc.tile_pool(name="zpool", bufs=4) as pz,
        tc.tile_pool(name="persist", bufs=1) as pp,
    ):
        wb_t = pp.tile([p_dim, f_dim + 1], dt.float32)
        nc.sync.dma_start(out=wb_t[:], in_=wb_d[:])
        bvec = wb_t[:, f_dim:f_dim + 1]
        # replicate the W row to [p, i_dim*f] with unit-stride doubling
        # copies (the copy chain also makes DVE observe the wb DMA lane, so
        # later consumers carry no extra sync wait)
        wrep_t = pp.tile([p_dim, seg], dt.float32)
        nc.vector.tensor_copy(wrep_t[:, 0:f_dim], wb_t[:, 0:f_dim])
        k = f_dim
        while k < seg:
            n = min(k, seg - k)
            nc.vector.tensor_copy(wrep_t[:, k:k + n], wrep_t[:, 0:n])
            k += n
        wrep = wrep_t[:]
        prime_t = pp.tile([p_dim, 1], dt.float32)
        nc.vector.tensor_copy(prime_t[:], wb_t[:, 0:1])
        # touch Exp early so the ACT table set (exp+ln) loads during the
        # compute phase instead of in the kernel tail
        warm_t = pp.tile([p_dim, 1], dt.float32)
        nc.scalar.activation(warm_t[:], prime_t[:], Act.Exp)
        # DVE-written and ACT-written accumulators are separate tiles so each
        # output DMA carries exactly one wait
        acc_t = pp.tile([p_dim, 4], dt.float32)
        accsp_t = pp.tile([p_dim, 1], dt.float32)

        z_all = pp.tile([p_dim, bsh, s_dim, i_dim], dt.float32)
        # quarter the very first chunk, first two quarters on HWDGE (lower
        # fixed latency, issued right after the tiny wb load) so the DVE
        # starts ~2.5us after kernel start instead of ~6.5us
        q = seg // 4
        iq = i_dim // 4
        xc0 = px.tile([p_dim, seg], dt.float32, tag="x")
        nc.sync.dma_start(out=xc0[:, 0:q], in_=x_re[0, 0][:, 0:q])
        nc.sync.dma_start(out=xc0[:, q:2 * q], in_=x_re[0, 0][:, q:2 * q])
        nc.gpsimd.dma_start(out=xc0[:, 2 * q:3 * q], in_=x_re[0, 0][:, 2 * q:3 * q])
        nc.gpsimd.dma_start(out=xc0[:, 3 * q:seg], in_=x_re[0, 0][:, 3 * q:seg])
        lab_t = pp.tile([p_dim, bsh, s_dim, i_dim], dt.float32)
        nc.sync.dma_start(out=lab_t[:], in_=lab_re)
        lab2 = None  # defined in the z stage below
        for b in range(bsh):
            jn = None
            if b >= 2:
                join_t = pz.tile([p_dim, s_dim * i_dim], dt.float32, tag="join")
                jn = nc.gpsimd.tensor_copy(
                    join_t[:], z_all[:, b - 2].rearrange("p s i -> p (s i)"))
            for s in range(s_dim):
                if b == 0 and s == 0:
                    for h in range(4):
                        sl = slice(h * q, (h + 1) * q)
                        nc.vector.tensor_tensor(xc0[:, sl], xc0[:, sl],
                                                wrep_t[:, sl], Alu.mult)
                        nc.vector.tensor_reduce(
                            z_all[:, 0, 0, h * iq:(h + 1) * iq],
                            xc0[:, sl].rearrange("p (i f) -> p i f", f=f_dim),
                            axis=Ax.X, op=Alu.add)
                    continue
                xc = px.tile([p_dim, seg], dt.float32, tag="x")
                xl = nc.gpsimd.dma_start(out=xc[:], in_=x_re[b, s])
                if jn is not None:
                    add_dep_helper(xl.ins, jn.ins, sync=False,
                                   reason="x load after WAR-carrier join")
                nc.vector.tensor_tensor(xc[:], xc[:], wrep, Alu.mult)
                nc.vector.tensor_reduce(
                    z_all[:, b, s],
                    xc[:].rearrange("p (i f) -> p i f", f=f_dim),
                    axis=Ax.X, op=Alu.add)

        # ---- z stage, batched over all batches: [p, bsh*s*i] views ----
        fr_all = bsh * s_dim * i_dim
        z2 = z_all[:].rearrange("p b s i -> p (b s i)")
        nc.vector.tensor_scalar(z2, z2, bvec, None, Alu.add)
        lab2 = lab_t[:].rearrange("p b s i -> p (b s i)")

        pred_t = pp.tile([p_dim, fr_all], dt.float32)
        nc.vector.tensor_scalar(pred_t[:], z2, 0.0, None, Alu.is_gt)
        ne_t = pp.tile([p_dim, fr_all], dt.float32)
        nc.vector.tensor_tensor(ne_t[:], lab2, pred_t[:], Alu.not_equal)

        # per-frame sums over s (s innermost in strided views)
        bi = bsh * i_dim
        nesum_t = pp.tile([p_dim, bi], dt.float32)
        nc.vector.tensor_reduce(
            nesum_t[:], ne_t[:].rearrange("p (b s i) -> p b i s", b=bsh, s=s_dim),
            axis=Ax.X, op=Alu.add)
        lsum_t = pp.tile([p_dim, bi], dt.float32)
        nc.vector.tensor_reduce(
            lsum_t[:], lab_t[:].rearrange("p b s i -> p b i s"),
            axis=Ax.X, op=Alu.add)
        psum_t = pp.tile([p_dim, bi], dt.float32)
        nc.vector.tensor_reduce(
            psum_t[:], pred_t[:].rearrange("p (b s i) -> p b i s", b=bsh, s=s_dim),
            axis=Ax.X, op=Alu.add)

        lz_t = pp.tile([p_dim, bi], dt.float32)
        nc.vector.tensor_scalar(lz_t[:], lsum_t[:], 0.5, None, Alu.is_lt)
        pz_t = pp.tile([p_dim, bi], dt.float32)
        nc.vector.tensor_scalar(pz_t[:], psum_t[:], 0.5, None, Alu.is_lt)

        # correct = sum(nesum < 0.5)
        scr_t = pp.tile([p_dim, bi], dt.float32)
        nc.vector.tensor_scalar(
            scr_t[:], nesum_t[:], 0.5, None, Alu.is_lt, Alu.add,
            accum_out=acc_t[:, c_corr:c_corr + 1])
        # FA = sum((nesum >= 0.5) * label_zero)
        scr2_t = pp.tile([p_dim, bi], dt.float32)
        nc.vector.scalar_tensor_tensor(
            scr2_t[:], nesum_t[:], 0.5, lz_t[:], Alu.is_ge, Alu.mult,
            accum_out=acc_t[:, c_fa:c_fa + 1])
        # MS = sum((nesum >= 0.5) * (lsum >= 0.5) * pred_zero)
        t_t = pp.tile([p_dim, bi], dt.float32)
        nc.vector.scalar_tensor_tensor(
            t_t[:], lsum_t[:], 0.5, pz_t[:], Alu.is_ge, Alu.mult)
        scr3_t = pp.tile([p_dim, bi], dt.float32)
        nc.vector.scalar_tensor_tensor(
            scr3_t[:], nesum_t[:], 0.5, t_t[:], Alu.is_ge, Alu.mult,
            accum_out=acc_t[:, c_ms:c_ms + 1])

        # z*y
        zys_t = pp.tile([p_dim, fr_all], dt.float32)
        nc.vector.scalar_tensor_tensor(
            zys_t[:], z2, 1.0, lab2, Alu.mult, Alu.mult,
            accum_out=acc_t[:, c_zy:c_zy + 1])

        # softplus = ln(1 + exp(z)); |z| <= ~4 so exp can't overflow
        e_t = pp.tile([p_dim, fr_all], dt.float32)
        nc.scalar.activation(e_t[:], z2, Act.Exp)
        sp_t = pp.tile([p_dim, fr_all], dt.float32)
        nc.scalar.activation(
            sp_t[:], e_t[:], Act.Ln, bias=1.0,
            accum_out=accsp_t[:, 0:1])

        nc.sync.dma_start(out=acc_d[:, 1:5], in_=acc_t[:])
        nc.sync.dma_start(out=acc_d[:, 0:1], in_=accsp_t[:])
    nc.finalize()
    return nc


_CACHE = {}


def _get_nc():
    if "nc" not in _CACHE:
        _CACHE["nc"] = build_nc()
    return _CACHE["nc"]


def _host_inputs(W, b):
    wrow = np.asarray(W, np.float32).reshape(-1)  # [F]
    bval = np.float32(np.asarray(b, np.float32).reshape(-1)[0])
    wb = np.empty((P, F + 1), np.float32)
    wb[:, :F] = wrow[None, :]
    wb[:, F] = bval
    return wb


def finalize(acc_sum):
    """acc_sum: float64 [ACC_COLS-wise] summed over cores+partitions+b."""
    sp = float(acc_sum[C_SP])
    zy = float(acc_sum[C_ZY])
    correct = float(acc_sum[C_CORR])
    FA = float(acc_sum[C_FA])
    MS = float(acc_sum[C_MS])

    Ssum = sp - zy
    BT = float(B * T)
    total_loss = Ssum / BT + Ssum / 4.0
    loss = total_loss / BT

    # replicate the reference's sequential fp32 normalization bit-exactly
    f = np.float32
    correct, FA, MS, BT32 = f(correct), f(FA), f(MS), f(BT)
    SC = f(f(f(BT32 - correct) - FA) - MS)
    DER = f(f(f(f(MS + FA) + SC)) / f(f(f(MS + FA) + SC) + correct))
    MS = f(MS / f(f(f(MS + FA) + SC) + correct))
    FA = f(FA / f(f(f(MS + FA) + SC) + correct))
    SC = f(SC / f(f(f(MS + FA) + SC) + correct))
    return (
        np.array(loss, dtype=np.float32),
        np.array(DER, dtype=np.float32),
        np.array(MS, dtype=np.float32),
        np.array(FA, dtype=np.float32),
        np.array(SC, dtype=np.float32),
    )


def kernel(x, labels, W, b):
    from concourse.bass_utils import run_bass_kernel_spmd

    x = np.ascontiguousarray(np.asarray(x, np.float32))
    labels = np.ascontiguousarray(np.asarray(labels, np.float32))
    wb = _host_inputs(W, b)

    nc = _get_nc()
    in_maps = []
    for c in range(NCORES):
        in_maps.append({
            "x": x[c * BSH:(c + 1) * BSH],
            "labels": labels[c * BSH:(c + 1) * BSH],
            "wb": wb,
        })
    res = run_bass_kernel_spmd(nc, in_maps, list(range(NCORES)), trace=TRACE)
    LAST_RESULT[0] = res
    acc = np.stack([np.asarray(r["acc_out"], np.float64) for r in res.results])
    acc_sum = acc.sum(axis=(0, 1))  # [ACC_COLS]
    return finalize(acc_sum)



# revision 45
# speedup vs baseline: 1.3995x; 1.3995x over previous
"""Trainium2 Bass kernel for nn_Loss_60430189855357.

BCEWithLogits loss + frame metrics over x[32,4,4000,96] @ W[96] + b.

Strategy (data-parallel over batch, 8 cores):
  - each core gets x[4,4,4000,96] and labels[4,4,4000]
  - x streams in per (b, s) chunk (1.5 MB) over SP/HWDGE; ACT casts
    fp32 -> fp16 (Copy)
  - DVE computes xw = x16 * Wrep16 at the 2x fp16 rate and folds f
    96->24 by in-place pairwise adds; Pool folds 24->6 into a persistent
    c6 tile; a per-chunk DVE reduce folds 6->1 into z (fp32)
  - chunk order puts every batch's s=3 chunk at the stream end so the
    per-batch metric finals interleave across the tail instead of
    bunching after the last chunk
  - per-chunk stage: pred/ne/zy-accum plus exp (ACT) and group products
    of (1+e^z); ln of the products happens on the HOST, so the ACT
    engine only ever runs Copy/Exp (one act-table set, zero reloads)
  - per-core output: [125, 92] = zy per (b,s) [16] + corr/fa/ms per b
    [12] + softplus group products [64]; host takes ln, reduces, and
    applies the reference's sequential normalization.
"""

import os
import sys

import numpy as np

if os.path.isdir("/opt/trn_rl_repo") and "/opt/trn_rl_repo" not in sys.path:
    sys.path.insert(0, "/opt/trn_rl_repo")

B, S, T, F = 32, 4, 4000, 96
NCORES = 8
BSH = B // NCORES  # 4 batches per core
P = 125            # SBUF partitions used (T = P * I)
I = T // P         # 32 t-rows per partition
SEG = I * F        # 3072 contiguous floats per (partition, s)
PG = 8             # elements per softplus product group
NG = I // PG       # product groups per (b, s) chunk
ACC_COLS = BSH * S + BSH * 3 + BSH * S * NG  # 16 zy + 12 metrics + 64 prods

TRACE = False          # test.py can flip this to get a profiled run
LAST_RESULT = [None]   # test.py reads BassKernelResults from here


def build_nc(bsh=BSH, s_dim=S, t_dim=T, f_dim=F, p_dim=P):
    import concourse.bacc as bacc
    import concourse.mybir as mybir
    from concourse.tile import TileContext
    from concourse.tile_rust import add_dep_helper

    i_dim = t_dim // p_dim
    assert p_dim * i_dim == t_dim
    seg = i_dim * f_dim
    dt = mybir.dt
    Alu = mybir.AluOpType
    Ax = mybir.AxisListType
    Act = mybir.ActivationFunctionType

    nc = bacc.Bacc()
    x_d = nc.declare_dram_parameter("x", [bsh, s_dim, t_dim, f_dim], dt.float32, isOutput=False)
    lab_d = nc.declare_dram_parameter("labels", [bsh, s_dim, t_dim], dt.float32, isOutput=False)
    wb_d = nc.declare_dram_parameter("wb", [p_dim, f_dim + 1], dt.float32, isOutput=False)
    acc_d = nc.declare_dram_parameter("acc_out", [p_dim, ACC_COLS], dt.float32, isOutput=True)

    # partition p owns t-rows [i_dim*p, i_dim*(p+1))
    x_re = x_d[:].rearrange("b s (p i) f -> b s p (i f)", p=p_dim)
    lab_re = lab_d[:].rearrange("b s (p i) -> p b s i", p=p_dim)

    # chunk order: each batch's s<3 run is followed by one of b3's chunks and
    # the batch's own s=3 chunk, so metric finals spread across the stream and
    # only b3's final lands at the very end
    chunks = []
    for b in range(bsh - 1):
        chunks += [(b, s) for s in range(s_dim - 1)]
        chunks += [(bsh - 1, b), (b, s_dim - 1)]
    chunks += [(bsh - 1, s_dim - 1)]

    # pieces: first and last chunks quartered, second-to-last halved
    pieces = []
    chunk_last_piece = {}
    for ci, (b, s) in enumerate(chunks):
        if ci == 0 or ci == len(chunks) - 1:
            iq = i_dim // 4
            for h in range(4):
                pieces.append((b, s, h * iq, (h + 1) * iq))
        elif ci == len(chunks) - 2:
            ih = i_dim // 2
            for h in range(2):
                pieces.append((b, s, h * ih, (h + 1) * ih))
        else:
            pieces.append((b, s, 0, i_dim))
        chunk_last_piece[(b, s)] = len(pieces) - 1

    with (
        TileContext(nc) as tc,
        tc.tile_pool(name="xpool", bufs=8) as px,
        tc.tile_pool(name="fpool", bufs=6) as pf,
        tc.tile_pool(name="bpool", bufs=3) as pb,
        tc.tile_pool(name="c6pool", bufs=5) as pc,
        tc.tile_pool(name="persist", bufs=1) as pp,
        nc.allow_low_precision(reason="fp16 product tree; validated 8e-5 max rel err"),
    ):
        # first x piece ahead of everything so the DMA stream starts earliest
        b0, s0, i00, i01 = pieces[0]
        xc0 = px.tile([p_dim, seg], dt.float32, tag="x")
        n0 = (i01 - i00) * f_dim
        nc.sync.dma_start(out=xc0[:, 0:n0], in_=x_re[b0, s0][:, i00 * f_dim:i01 * f_dim])

        wb_t = pp.tile([p_dim, f_dim + 1], dt.float32)
        nc.sync.dma_start(out=wb_t[:], in_=wb_d[:])
        bvec = wb_t[:, f_dim:f_dim + 1]
        negb_t = pp.tile([p_dim, 1], dt.float32)
        nc.vector.tensor_scalar(negb_t[:], bvec, -1.0, None, Alu.mult)
        # fp16 W replicated to [p, i*f] with unit-stride doubling copies
        wrep_t = pp.tile([p_dim, seg], dt.float16)
        nc.vector.tensor_copy(wrep_t[:, 0:f_dim], wb_t[:, 0:f_dim])
        k = f_dim
        while k < seg:
            n = min(k, seg - k)
            nc.vector.tensor_copy(wrep_t[:, k:k + n], wrep_t[:, 0:n])
            k += n
        # touch Exp early so the ACT table set (exp+copy) loads during the
        # compute phase instead of on the first cast
        warm_t = pp.tile([p_dim, 1], dt.float32)
        nc.scalar.activation(warm_t[:], bvec, Act.Exp)

        # acc columns: [0..15] zy per (b,s); [16..27] corr/fa/ms per b;
        # [28..91] softplus group products — one tile so one store suffices
        acc_t = pp.tile([p_dim, ACC_COLS], dt.float32)
        prod_t = acc_t[:, bsh * s_dim + bsh * 3:ACC_COLS]
        lab_t = pp.tile([p_dim, bsh, s_dim, i_dim], dt.float32)
        lsum_t = pp.tile([p_dim, bsh, i_dim], dt.float32)
        lz_t = pp.tile([p_dim, bsh, i_dim], dt.float32)
        # running max over s of z and of (pred != label); the finals only need
        # thresholds of these (all-match, any-mismatch, pred-all-zero)
        zmax_t = pp.tile([p_dim, bsh, i_dim], dt.float32)
        nemax_t = pp.tile([p_dim, bsh, i_dim], dt.float32)

        # per-chunk 6-wide partial-sum tiles; pooled so each chunk-stage read
        # depends only on its own chunk's tree writes (tile-granularity deps)
        chunk_c6 = {}

        def emit_piece(b, s, i0, i1, xc=None, last=False):
            n_i = i1 - i0
            n = n_i * f_dim
            if xc is None:
                xc = px.tile([p_dim, seg], dt.float32, tag="x")
                src = x_re[b, s][:, i0 * f_dim:i1 * f_dim]
                nc.sync.dma_start(out=xc[:, 0:n], in_=src)
            fc = pf.tile([p_dim, seg], dt.float16, tag="f")
            cast_op = nc.scalar.activation(fc[:, 0:n], xc[:, 0:n], Act.Copy)
            f3 = fc[:, 0:n].rearrange("p (i f) -> p i f", f=f_dim)
            mult_op = nc.vector.tensor_tensor(
                f3[:, :, 0:f_dim], f3[:, :, 0:f_dim],
                wrep_t[:, 0:n].rearrange("p (i f) -> p i f", f=f_dim),
                Alu.mult)
            nc.vector.tensor_tensor(f3[:, :, 0:48], f3[:, :, 0:48], f3[:, :, 48:96], Alu.add)
            # near the stream end Pool has slack and DVE is the critical chain
            lvl2_eng = nc.gpsimd if last else nc.vector
            lvl2_eng.tensor_tensor(f3[:, :, 0:24], f3[:, :, 0:24], f3[:, :, 24:48], Alu.add)
            if (b, s) not in chunk_c6:
                c6 = pc.tile([p_dim, i_dim, 6], dt.float16, tag="c6", name=f"c6_{b}_{s}")
                chunk_c6[(b, s)] = c6
            c6 = chunk_c6[(b, s)]
            nc.gpsimd.tensor_tensor(f3[:, :, 0:12], f3[:, :, 0:12], f3[:, :, 12:24], Alu.add)
            nc.gpsimd.tensor_tensor(c6[:, i0:i1], f3[:, :, 0:6], f3[:, :, 6:12], Alu.add)
            return cast_op, mult_op

        deferred_sp = []

        def emit_softplus(b, s, zb, anchors=None):
            e_t = pb.tile([p_dim, i_dim], dt.float32, tag="e")
            exp_op = nc.scalar.activation(e_t[:], zb[:], Act.Exp, bias=bvec)
            if anchors is not None:
                add_dep_helper(exp_op.ins, anchors[0].ins, sync=False,
                               reason="exp after current chunk's cast")
            nc.scalar.activation(e_t[:], e_t[:], Act.Copy, bias=1.0)
            g0 = (b * s_dim + s) * NG
            nc.vector.tensor_reduce(
                prod_t[:, g0:g0 + NG],
                e_t[:].rearrange("p (g e) -> p g e", e=PG),
                axis=Ax.X, op=Alu.mult)

        def emit_cstage(b, s, anchors=None, defer_sp=False):
            # per-chunk stage: z, mismatch, zy accum, exp, 1+e, group products.
            # nosync anchors keep the scheduler from gluing the stage right
            # after its Pool producer (which would stall the in-order DVE).
            zb = pb.tile([p_dim, i_dim], dt.float32, tag="zb", bufs=8)
            zb_op = nc.vector.tensor_reduce(
                zb[:], chunk_c6.pop((b, s))[:], axis=Ax.X, op=Alu.add)
            if anchors is not None:
                add_dep_helper(zb_op.ins, anchors[1].ins, sync=False,
                               reason="consume c6 after current chunk's mult")
            # ne = (z > -bias) != label, folded into one op
            ne = pb.tile([p_dim, i_dim], dt.float32, tag="ne")
            nc.vector.scalar_tensor_tensor(
                ne[:], zb[:], negb_t[:], lab_t[:, b, s], Alu.is_gt, Alu.not_equal)
            if s == 0:
                nc.vector.tensor_copy(nemax_t[:, b], ne[:])
                nc.vector.tensor_copy(zmax_t[:, b], zb[:])
            else:
                nc.vector.tensor_tensor(nemax_t[:, b], nemax_t[:, b], ne[:], Alu.max)
                nc.vector.tensor_tensor(zmax_t[:, b], zmax_t[:, b], zb[:], Alu.max)
            zys = pb.tile([p_dim, i_dim], dt.float32, tag="zys")
            nc.vector.scalar_tensor_tensor(
                zys[:], zb[:], bvec, lab_t[:, b, s], Alu.add, Alu.mult,
                accum_out=acc_t[:, b * s_dim + s:b * s_dim + s + 1])
            if defer_sp:
                # keep the tail's ACT sequencer free of data-blocked exps:
                # softplus parts of late chunks run after the final cast
                deferred_sp.append((b, s, zb))
            else:
                emit_softplus(b, s, zb, anchors=anchors)

        def emit_bfinal(b):
            # per-batch metric final from the running maxes:
            #   all-match = nemax < 0.5; pred-all-zero = zmax <= -bias
            pz = pb.tile([p_dim, i_dim], dt.float32, tag="pz")
            nc.vector.tensor_scalar(pz[:], zmax_t[:, b], negb_t[:], None, Alu.is_le)
            mcol = bsh * s_dim + b * 3
            s1 = pb.tile([p_dim, i_dim], dt.float32, tag="s1")
            nc.vector.tensor_scalar(
                s1[:], nemax_t[:, b], 0.5, None, Alu.is_lt, Alu.add,
                accum_out=acc_t[:, mcol:mcol + 1])
            s2 = pb.tile([p_dim, i_dim], dt.float32, tag="s2")
            nc.vector.scalar_tensor_tensor(
                s2[:], nemax_t[:, b], 0.5, lz_t[:, b], Alu.is_ge, Alu.mult,
                accum_out=acc_t[:, mcol + 1:mcol + 2])
            t_t = pb.tile([p_dim, i_dim], dt.float32, tag="t")
            nc.vector.scalar_tensor_tensor(
                t_t[:], lsum_t[:, b], 0.5, pz[:], Alu.is_ge, Alu.mult)
            s3 = pb.tile([p_dim, i_dim], dt.float32, tag="s3")
            nc.vector.scalar_tensor_tensor(
                s3[:], nemax_t[:, b], 0.5, t_t[:], Alu.is_ge, Alu.mult,
                accum_out=acc_t[:, mcol + 2:mcol + 3])

        # chunk stages run ~1 chunk after their data so cross-engine waits are
        # already satisfied; each batch's final follows its s=3 chunk stage
        npieces = len(pieces)
        stage_after = {}
        for ci, (b, s) in enumerate(chunks):
            lastp = chunk_last_piece[(b, s)]
            trigger = min(lastp + 2, npieces - 1)
            stage_after.setdefault(trigger, []).append((b, s))
        labels_after = 2

        for j, (b, s, i0, i1) in enumerate(pieces):
            anchors = emit_piece(b, s, i0, i1, xc=xc0 if j == 0 else None,
                                 last=(j >= npieces - 6))
            if j == labels_after:
                nc.sync.dma_start(out=lab_t[:], in_=lab_re)
                nc.vector.tensor_reduce(
                    lsum_t[:], lab_t[:].rearrange("p b s i -> p b i s"),
                    axis=Ax.X, op=Alu.add)
                nc.vector.tensor_scalar(
                    lz_t[:].rearrange("p b i -> p (b i)"),
                    lsum_t[:].rearrange("p b i -> p (b i)"),
                    0.5, None, Alu.is_lt)
            for (cb, cs) in stage_after.get(j, []):
                emit_cstage(cb, cs, anchors=anchors,
                            defer_sp=(j >= npieces - 7))
                if cs == s_dim - 1:
                    emit_bfinal(cb)

        for (db, ds, dzb) in deferred_sp:
            emit_softplus(db, ds, dzb)

        # single deferred store on HWDGE; generation starts as soon as the
        # last accumulator/product write completes
        nc.sync.dma_start(out=acc_d[:], in_=acc_t[:])
    nc.finalize()
    return nc


_CACHE = {}


def _get_nc():
    if "nc" not in _CACHE:
        _CACHE["nc"] = build_nc()
    return _CACHE["nc"]


def _host_inputs(W, b):
    wrow = np.asarray(W, np.float32).reshape(-1)  # [F]
    bval = np.float32(np.asarray(b, np.float32).reshape(-1)[0])
    wb = np.empty((P, F + 1), np.float32)
    wb[:, :F] = wrow[None, :]
    wb[:, F] = bval
    return wb


def finalize(acc_sum):
    """acc_sum: float64 [sp, zy, correct, FA, MS] summed over cores+partitions+b."""
    sp = float(acc_sum[0])
    zy = float(acc_sum[1])
    correct = float(acc_sum[2])
    FA = float(acc_sum[3])
    MS = float(acc_sum[4])

    Ssum = sp - zy
    BT = float(B * T)
    total_loss = Ssum / BT + Ssum / 4.0
    loss = total_loss / BT

    # replicate the reference's sequential fp32 normalization bit-exactly
    f = np.float32
    correct, FA, MS, BT32 = f(correct), f(FA), f(MS), f(BT)
    SC = f(f(f(BT32 - correct) - FA) - MS)
    DER = f(f(f(f(MS + FA) + SC)) / f(f(f(MS + FA) + SC) + correct))
    MS = f(MS / f(f(f(MS + FA) + SC) + correct))
    FA = f(FA / f(f(f(MS + FA) + SC) + correct))
    SC = f(SC / f(f(f(MS + FA) + SC) + correct))
    return (
        np.array(loss, dtype=np.float32),
        np.array(DER, dtype=np.float32),
        np.array(MS, dtype=np.float32),
        np.array(FA, dtype=np.float32),
        np.array(SC, dtype=np.float32),
    )


def kernel(x, labels, W, b):
    from concourse.bass_utils import run_bass_kernel_spmd

    x = np.ascontiguousarray(np.asarray(x, np.float32))
    labels = np.ascontiguousarray(np.asarray(labels, np.float32))
    wb = _host_inputs(W, b)

    nc = _get_nc()
    in_maps = []
    for c in range(NCORES):
        in_maps.append({
            "x": x[c * BSH:(c + 1) * BSH],
            "labels": labels[c * BSH:(c + 1) * BSH],
            "wb": wb,
        })
    res = run_bass_kernel_spmd(nc, in_maps, list(range(NCORES)), trace=TRACE)
    LAST_RESULT[0] = res
    acc = np.stack([np.asarray(r["acc_out"], np.float64) for r in res.results])
    nzy = BSH * S
    nmet = BSH * 3
    zy = acc[:, :, 0:nzy].sum()
    met = acc[:, :, nzy:nzy + nmet].reshape(NCORES, P, BSH, 3).sum(axis=(0, 1, 2))
    prods = acc[:, :, nzy + nmet:]
    sp = np.log(prods).sum()
    acc_sum = np.array([sp, zy, met[0], met[1], met[2]], np.float64)
    return finalize(acc_sum)


# revision 80
# speedup vs baseline: 1.4070x; 1.0054x over previous
"""Trainium2 Bass kernel for nn_Loss_60430189855357.

BCEWithLogits loss + frame metrics over x[32,4,4000,96] @ W[96] + b.

Strategy (data-parallel over batch, 8 cores):
  - each core gets x[4,4,4000,96] and labels[4,4,4000]
  - x streams in per (b, s) chunk (1.5 MB) over SP/HWDGE; ACT casts
    fp32 -> fp16 (Copy)
  - DVE computes xw = x16 * Wrep16 at the 2x fp16 rate and folds f
    96->24 by in-place pairwise adds; Pool folds 24->6 into a persistent
    c6 tile; a per-chunk DVE reduce folds 6->1 into z (fp32)
  - chunk order puts every batch's s=3 chunk at the stream end so the
    per-batch metric finals interleave across the tail instead of
    bunching after the last chunk
  - per-chunk stage: pred/ne/zy-accum plus exp (ACT) and group products
    of (1+e^z); ln of the products happens on the HOST, so the ACT
    engine only ever runs Copy/Exp (one act-table set, zero reloads)
  - per-core output: [125, 92] = zy per (b,s) [16] + corr/fa/ms per b
    [12] + softplus group products [64]; host takes ln, reduces, and
    applies the reference's sequential normalization.
"""

import os
import sys

import numpy as np

if os.path.isdir("/opt/trn_rl_repo") and "/opt/trn_rl_repo" not in sys.path:
    sys.path.insert(0, "/opt/trn_rl_repo")

B, S, T, F = 32, 4, 4000, 96
NCORES = 8
BSH = B // NCORES  # 4 batches per core
P = 125            # SBUF partitions used (T = P * I)
I = T // P         # 32 t-rows per partition
SEG = I * F        # 3072 contiguous floats per (partition, s)
PG = 8             # elements per softplus product group
NG = I // PG       # product groups per (b, s) chunk
NZY = 19           # zy partial columns (13 whole chunks + 3x2 tail slices)
NFIN = 6           # metric final slices (b0, b1 whole + b2 x2 + b3 x2)
NPROD = BSH * S * NG
NZRAW = 160        # raw z columns for late chunks (softplus done on host)
# layout: [zy x NZY][corr x NFIN][fa x NFIN][ms x NFIN][prods x NPROD][z x NZRAW]
ACC_COLS = NZY + 3 * NFIN + NPROD + NZRAW

TRACE = False          # test.py can flip this to get a profiled run
LAST_RESULT = [None]   # test.py reads BassKernelResults from here


def build_nc(bsh=BSH, s_dim=S, t_dim=T, f_dim=F, p_dim=P):
    import concourse.bacc as bacc
    import concourse.mybir as mybir
    from concourse.tile import TileContext
    from concourse.tile_rust import add_dep_helper

    i_dim = t_dim // p_dim
    assert p_dim * i_dim == t_dim
    seg = i_dim * f_dim
    dt = mybir.dt
    Alu = mybir.AluOpType
    Ax = mybir.AxisListType
    Act = mybir.ActivationFunctionType

    nc = bacc.Bacc()
    x_d = nc.declare_dram_parameter("x", [bsh, s_dim, t_dim, f_dim], dt.float32, isOutput=False)
    lab_d = nc.declare_dram_parameter("labels", [bsh, s_dim, t_dim], dt.float32, isOutput=False)
    wb_d = nc.declare_dram_parameter("wb", [p_dim, f_dim + 1], dt.float32, isOutput=False)
    acc_d = nc.declare_dram_parameter("acc_out", [p_dim, ACC_COLS], dt.float32, isOutput=True)

    # partition p owns t-rows [i_dim*p, i_dim*(p+1))
    x_re = x_d[:].rearrange("b s (p i) f -> b s p (i f)", p=p_dim)
    lab_re = lab_d[:].rearrange("b s (p i) -> p b s i", p=p_dim)

    # chunk order: each batch's s<3 run is followed by one of b3's chunks and
    # the batch's own s=3 chunk, so metric finals spread across the stream and
    # only b3's final lands at the very end
    chunks = []
    for b in range(bsh - 1):
        chunks += [(b, s) for s in range(s_dim - 1)]
        chunks += [(bsh - 1, b), (b, s_dim - 1)]
    chunks += [(bsh - 1, s_dim - 1)]

    # pieces: the first chunk is quartered for an early pipeline start; the
    # DMA stream ends with the FINAL quarters of the last three chunks, so
    # 3/4 of each tail chunk's compute lands earlier and the post-stream
    # critical chain is one quarter-piece long. Stage slices follow pieces.
    iq = i_dim // 4
    i34 = 3 * iq
    tail3 = chunks[-3:]
    pieces = []
    for ci, (b, s) in enumerate(chunks):
        if ci == 0:
            for h in range(4):
                pieces.append((b, s, h * iq, (h + 1) * iq))
        elif (b, s) in tail3:
            pieces.append((b, s, 0, i34))
        else:
            pieces.append((b, s, 0, i_dim))
    for (b, s) in tail3:
        pieces.append((b, s, i34, i_dim))

    with (
        TileContext(nc) as tc,
        tc.tile_pool(name="xpool", bufs=8) as px,
        tc.tile_pool(name="fpool", bufs=6) as pf,
        tc.tile_pool(name="bpool", bufs=3) as pb,
        tc.tile_pool(name="c6pool", bufs=8) as pc,
        tc.tile_pool(name="persist", bufs=1) as pp,
        nc.allow_low_precision(reason="fp16 product tree; validated 8e-5 max rel err"),
    ):
        # first x piece ahead of everything so the DMA stream starts earliest
        b0, s0, i00, i01 = pieces[0]
        xc0 = px.tile([p_dim, seg], dt.float32, tag="x")
        n0 = (i01 - i00) * f_dim
        nc.sync.dma_start(out=xc0[:, 0:n0], in_=x_re[b0, s0][:, i00 * f_dim:i01 * f_dim])

        wb_t = pp.tile([p_dim, f_dim + 1], dt.float32)
        nc.sync.dma_start(out=wb_t[:], in_=wb_d[:])
        bvec = wb_t[:, f_dim:f_dim + 1]
        negb_t = pp.tile([p_dim, 1], dt.float32)
        nc.vector.tensor_scalar(negb_t[:], bvec, -1.0, None, Alu.mult)
        # fp16 W replicated to [p, i*f] with unit-stride doubling copies
        wrep_t = pp.tile([p_dim, seg], dt.float16)
        nc.vector.tensor_copy(wrep_t[:, 0:f_dim], wb_t[:, 0:f_dim])
        k = f_dim
        while k < seg:
            n = min(k, seg - k)
            nc.vector.tensor_copy(wrep_t[:, k:k + n], wrep_t[:, 0:n])
            k += n
        # touch Exp early so the ACT table set (exp+copy) loads during the
        # compute phase instead of on the first cast
        warm_t = pp.tile([p_dim, 1], dt.float32)
        nc.scalar.activation(warm_t[:], bvec, Act.Exp)

        # block-column accumulators (host sums each block); one tile so a
        # single store suffices
        acc_t = pp.tile([p_dim, ACC_COLS], dt.float32)
        nc.vector.memset(acc_t[:], 0.0)
        prod_t = acc_t[:, NZY + 3 * NFIN:NZY + 3 * NFIN + NPROD]
        zraw_t = acc_t[:, NZY + 3 * NFIN + NPROD:ACC_COLS]
        zy_next = [0]
        fin_next = [0]
        zraw_next = [0]
        zraw_map = []  # (b, s, i0, i1, zraw col) for host-side softplus
        lab_t = pp.tile([p_dim, bsh, s_dim, i_dim], dt.float32)
        lsum_t = pp.tile([p_dim, bsh, i_dim], dt.float32)
        lz_t = pp.tile([p_dim, bsh, i_dim], dt.float32)
        # running max over s of z and of (pred != label); the finals only need
        # thresholds of these (all-match, any-mismatch, pred-all-zero)
        zmax_t = pp.tile([p_dim, bsh, i_dim], dt.float32)
        nemax_t = pp.tile([p_dim, bsh, i_dim], dt.float32)

        # per-chunk 6-wide partial-sum tiles; pooled so each chunk-stage read
        # depends only on its own chunk's tree writes (tile-granularity deps)
        chunk_c6 = {}

        def emit_piece(b, s, i0, i1, xc=None, last=False, tree_local=False):
            n_i = i1 - i0
            n = n_i * f_dim
            if xc is None:
                xc = px.tile([p_dim, seg], dt.float32, tag="x")
                src = x_re[b, s][:, i0 * f_dim:i1 * f_dim]
                nc.sync.dma_start(out=xc[:, 0:n], in_=src)
            fc = pf.tile([p_dim, seg], dt.float16, tag="f")
            cast_op = nc.scalar.activation(fc[:, 0:n], xc[:, 0:n], Act.Copy)
            f3 = fc[:, 0:n].rearrange("p (i f) -> p i f", f=f_dim)
            mult_op = nc.vector.tensor_tensor(
                f3[:, :, 0:f_dim], f3[:, :, 0:f_dim],
                wrep_t[:, 0:n].rearrange("p (i f) -> p i f", f=f_dim),
                Alu.mult)
            nc.vector.tensor_tensor(f3[:, :, 0:48], f3[:, :, 0:48], f3[:, :, 48:96], Alu.add)
            # Pool takes the lower tree mid-stream (keeps DVE duty low); the
            # final pieces keep the whole tree on DVE so the tail chain never
            # waits behind Pool's in-order queue
            tree_eng = nc.vector if tree_local else nc.gpsimd
            tree_eng.tensor_tensor(f3[:, :, 0:24], f3[:, :, 0:24], f3[:, :, 24:48], Alu.add)
            if (b, s) not in chunk_c6:
                c6 = pc.tile([p_dim, i_dim, 6], dt.float16, tag="c6", name=f"c6_{b}_{s}")
                chunk_c6[(b, s)] = c6
            c6 = chunk_c6[(b, s)]
            tree_eng.tensor_tensor(f3[:, :, 0:12], f3[:, :, 0:12], f3[:, :, 12:24], Alu.add)
            tree_eng.tensor_tensor(c6[:, i0:i1], f3[:, :, 0:6], f3[:, :, 6:12], Alu.add)
            return cast_op, mult_op

        def emit_softplus(b, s, i0, i1, zb, anchors=None):
            n_i = i1 - i0
            e_t = pb.tile([p_dim, i_dim], dt.float32, tag="e")
            exp_op = nc.scalar.activation(e_t[:, 0:n_i], zb, Act.Exp, bias=bvec)
            if anchors is not None:
                add_dep_helper(exp_op.ins, anchors[0].ins, sync=False,
                               reason="exp after current chunk's cast")
            nc.scalar.activation(e_t[:, 0:n_i], e_t[:, 0:n_i], Act.Copy, bias=1.0)
            g0 = (b * s_dim + s) * NG + i0 // PG
            nc.vector.tensor_reduce(
                prod_t[:, g0:g0 + n_i // PG],
                e_t[:, 0:n_i].rearrange("p (g e) -> p g e", e=PG),
                axis=Ax.X, op=Alu.mult)

        def emit_cstage(b, s, i0, i1, anchors=None, defer_sp=False):
            # per-slice stage: z, mismatch, running maxes, zy accum, softplus.
            # nosync anchors keep the scheduler from gluing the stage right
            # after its Pool producer (which would stall the in-order DVE).
            # Late slices skip on-device softplus: their z goes to DRAM raw
            # and the host computes ln(1+e^z) exactly.
            n_i = i1 - i0
            if defer_sp:
                zc = zraw_next[0]
                zraw_next[0] += n_i
                zraw_map.append((b, s, i0, i1, zc))
                zb = zraw_t[:, zc:zc + n_i]
            else:
                zbt = pb.tile([p_dim, i_dim], dt.float32, tag="zb", bufs=8)
                zb = zbt[:, 0:n_i]
            zb_op = nc.vector.tensor_reduce(
                zb, chunk_c6[(b, s)][:, i0:i1], axis=Ax.X, op=Alu.add)
            if i1 == i_dim:
                chunk_c6.pop((b, s))
            if anchors is not None:
                add_dep_helper(zb_op.ins, anchors[1].ins, sync=False,
                               reason="consume c6 after current chunk's mult")
            # ne = (z > -bias) != label, folded into one op
            ne = pb.tile([p_dim, i_dim], dt.float32, tag="ne")
            nc.vector.scalar_tensor_tensor(
                ne[:, 0:n_i], zb, negb_t[:], lab_t[:, b, s, i0:i1],
                Alu.is_gt, Alu.not_equal)
            if s == 0:
                nc.vector.tensor_copy(nemax_t[:, b, i0:i1], ne[:, 0:n_i])
                nc.vector.tensor_copy(zmax_t[:, b, i0:i1], zb)
            else:
                nc.vector.tensor_tensor(nemax_t[:, b, i0:i1], nemax_t[:, b, i0:i1],
                                        ne[:, 0:n_i], Alu.max)
                nc.vector.tensor_tensor(zmax_t[:, b, i0:i1], zmax_t[:, b, i0:i1],
                                        zb, Alu.max)
            zys = pb.tile([p_dim, i_dim], dt.float32, tag="zys")
            zcol = zy_next[0]
            zy_next[0] += 1
            nc.vector.scalar_tensor_tensor(
                zys[:, 0:n_i], zb, bvec, lab_t[:, b, s, i0:i1],
                Alu.add, Alu.mult, accum_out=acc_t[:, zcol:zcol + 1])
            if not defer_sp:
                emit_softplus(b, s, i0, i1, zb, anchors=anchors)

        def emit_bfinal(b, i0, i1):
            # per-batch metric final from the running maxes:
            #   all-match = nemax < 0.5; pred-all-zero = zmax <= -bias
            n_i = i1 - i0
            pz = pb.tile([p_dim, i_dim], dt.float32, tag="pz")
            nc.vector.tensor_scalar(pz[:, 0:n_i], zmax_t[:, b, i0:i1], negb_t[:],
                                    None, Alu.is_le)
            fcol = fin_next[0]
            fin_next[0] += 1
            s1 = pb.tile([p_dim, i_dim], dt.float32, tag="s1")
            nc.vector.tensor_scalar(
                s1[:, 0:n_i], nemax_t[:, b, i0:i1], 0.5, None, Alu.is_lt, Alu.add,
                accum_out=acc_t[:, NZY + fcol:NZY + fcol + 1])
            s2 = pb.tile([p_dim, i_dim], dt.float32, tag="s2")
            nc.vector.scalar_tensor_tensor(
                s2[:, 0:n_i], nemax_t[:, b, i0:i1], 0.5, lz_t[:, b, i0:i1],
                Alu.is_ge, Alu.mult,
                accum_out=acc_t[:, NZY + NFIN + fcol:NZY + NFIN + fcol + 1])
            t_t = pb.tile([p_dim, i_dim], dt.float32, tag="t")
            nc.vector.scalar_tensor_tensor(
                t_t[:, 0:n_i], lsum_t[:, b, i0:i1], 0.5, pz[:, 0:n_i],
                Alu.is_ge, Alu.mult)
            s3 = pb.tile([p_dim, i_dim], dt.float32, tag="s3")
            nc.vector.scalar_tensor_tensor(
                s3[:, 0:n_i], nemax_t[:, b, i0:i1], 0.5, t_t[:, 0:n_i],
                Alu.is_ge, Alu.mult,
                accum_out=acc_t[:, NZY + 2 * NFIN + fcol:NZY + 2 * NFIN + fcol + 1])

        # stage slices run ~1-2 pieces after their data so cross-engine waits
        # are already satisfied; batch finals follow their s=3 stage slices
        npieces = len(pieces)
        stage_after = {}
        for j, (b, s, i0, i1) in enumerate(pieces):
            if (b, s) == pieces[0][:2] and i1 != i_dim:
                continue  # first chunk staged whole at its last piece
            delta = 1 if i0 == 3 * (i_dim // 4) else 2
            trigger = min(j + delta, npieces - 1)
            stage_after.setdefault(trigger, []).append((b, s, i0, i1))
        labels_after = 2

        for j, (b, s, i0, i1) in enumerate(pieces):
            anchors = emit_piece(b, s, i0, i1, xc=xc0 if j == 0 else None,
                                 last=(j >= npieces - 5),
                                 tree_local=(j >= npieces - 3))
            if j == labels_after:
                nc.sync.dma_start(out=lab_t[:], in_=lab_re)
                nc.vector.tensor_reduce(
                    lsum_t[:], lab_t[:].rearrange("p b s i -> p b i s"),
                    axis=Ax.X, op=Alu.add)
                nc.vector.tensor_scalar(
                    lz_t[:].rearrange("p b i -> p (b i)"),
                    lsum_t[:].rearrange("p b i -> p (b i)"),
                    0.5, None, Alu.is_lt)
            for (cb, cs, ci0, ci1) in stage_after.get(j, []):
                cl0, cl1 = (ci0, ci1) if (cb, cs) in tail3 else (0, i_dim)
                emit_cstage(cb, cs, cl0, cl1, anchors=anchors,
                            defer_sp=(j >= npieces - 6))
                if cs == s_dim - 1:
                    emit_bfinal(cb, cl0, cl1)

        # single deferred store on HWDGE; generation starts as soon as the
        # last accumulator/product write completes
        nc.sync.dma_start(out=acc_d[:], in_=acc_t[:])
        assert zraw_next[0] <= NZRAW, zraw_next[0]
    nc.finalize()
    _CACHE["zraw_map"] = list(zraw_map)
    return nc


_CACHE = {}


def _get_nc():
    if "nc" not in _CACHE:
        _CACHE["nc"] = build_nc()
    return _CACHE["nc"]


def _host_inputs(W, b):
    wrow = np.asarray(W, np.float32).reshape(-1)  # [F]
    bval = np.float32(np.asarray(b, np.float32).reshape(-1)[0])
    wb = np.empty((P, F + 1), np.float32)
    wb[:, :F] = wrow[None, :]
    wb[:, F] = bval
    return wb


def finalize(acc_sum):
    """acc_sum: float64 [sp, zy, correct, FA, MS] summed over cores+partitions+b."""
    sp = float(acc_sum[0])
    zy = float(acc_sum[1])
    correct = float(acc_sum[2])
    FA = float(acc_sum[3])
    MS = float(acc_sum[4])

    Ssum = sp - zy
    BT = float(B * T)
    total_loss = Ssum / BT + Ssum / 4.0
    loss = total_loss / BT

    # replicate the reference's sequential fp32 normalization bit-exactly
    f = np.float32
    correct, FA, MS, BT32 = f(correct), f(FA), f(MS), f(BT)
    SC = f(f(f(BT32 - correct) - FA) - MS)
    DER = f(f(f(f(MS + FA) + SC)) / f(f(f(MS + FA) + SC) + correct))
    MS = f(MS / f(f(f(MS + FA) + SC) + correct))
    FA = f(FA / f(f(f(MS + FA) + SC) + correct))
    SC = f(SC / f(f(f(MS + FA) + SC) + correct))
    return (
        np.array(loss, dtype=np.float32),
        np.array(DER, dtype=np.float32),
        np.array(MS, dtype=np.float32),
        np.array(FA, dtype=np.float32),
        np.array(SC, dtype=np.float32),
    )


def kernel(x, labels, W, b):
    from concourse.bass_utils import run_bass_kernel_spmd

    x = np.ascontiguousarray(np.asarray(x, np.float32))
    labels = np.ascontiguousarray(np.asarray(labels, np.float32))
    wb = _host_inputs(W, b)

    nc = _get_nc()
    in_maps = []
    for c in range(NCORES):
        in_maps.append({
            "x": x[c * BSH:(c + 1) * BSH],
            "labels": labels[c * BSH:(c + 1) * BSH],
            "wb": wb,
        })
    res = run_bass_kernel_spmd(nc, in_maps, list(range(NCORES)), trace=TRACE)
    LAST_RESULT[0] = res
    acc = np.stack([np.asarray(r["acc_out"], np.float64) for r in res.results])
    zy = acc[:, :, 0:NZY].sum()
    corr = acc[:, :, NZY:NZY + NFIN].sum()
    fa = acc[:, :, NZY + NFIN:NZY + 2 * NFIN].sum()
    ms = acc[:, :, NZY + 2 * NFIN:NZY + 3 * NFIN].sum()
    nmeta = NZY + 3 * NFIN
    prods = acc[:, :, nmeta:nmeta + NPROD]
    zraw = acc[:, :, nmeta + NPROD:]
    # late chunks skipped on-device softplus; their prod groups are invalid
    # and their raw z columns carry the data instead
    mask = np.ones(NPROD, dtype=bool)
    for (mb, ms_, mi0, mi1, zc) in _CACHE["zraw_map"]:
        g0 = (mb * S + ms_) * NG + mi0 // PG
        mask[g0:g0 + (mi1 - mi0) // PG] = False
    bval = float(np.asarray(b, np.float32).reshape(-1)[0])
    sp = np.log(prods[:, :, mask]).sum()
    sp += np.logaddexp(0.0, zraw + bval).sum()
    acc_sum = np.array([sp, zy, corr, fa, ms], np.float64)
    return finalize(acc_sum)
